# revision 8
# baseline (speedup 1.0000x reference)
"""Trainium2 Bass kernel for 3-layer ConvLSTM2D stack + BN/ReLU + Conv3D+sigmoid.

Model (per reference):
  for l in 0..2:  h = bn_relu(conv_lstm(h, k_l, rk_l, b_l), g_l, be_l)
  y = sigmoid(conv3d(h, w3) + b3)

Shapes: x [B=8, T=12, 64, 64, 1], F=64 filters per layer, 3x3 kernels, SAME.

Sharding: data-parallel over B across 8 NeuronCores (1 image per core),
weights replicated.

Per-core layout: channels on SBUF partitions, pixels on the free axis in a
zero-padded 66x66 frame (flat 4356, stride 4360). A 3x3 conv is 9
shifted-tap matmuls accumulated in PSUM (K=chans, M=out-chans, N=pixels).
Taps are K-packed 2-per-matmul via shifted replicas on partitions 64-127 of
the moving tile:
  tile A = (src, src shifted +1)   -> tap pairs differing by 1
  tile B = (src, src shifted +66)  -> tap pairs differing by 66
All matmuls run in float32r (~13-bit mantissa, 1 cycle/row).

Recurrence per timestep: input-conv + recurrent-conv accumulate into the same
PSUM tiles (2 M-chunks of 128 out-chans x 8 N-chunks of 512 pixels), then
gates: sigmoid/tanh on ScalarE (conv bias folded in as per-partition bias),
c-state + h on VectorE; BN+ReLU is one ScalarE op (scale=g/sqrt(1+eps),
bias=be) writing the next layer's input frame, which round-trips through a
DRAM scratch buffer (SBUF can't hold a full layer sequence).

Conv3d: per input frame, 5 spatial tap-groups with lhsT [K,3] (3 dt columns)
-> partials P[3, chunk] in PSUM; staged to SBUF, scatter-DMA'd into a
[3T, 4096] ring (partition m*T + t), then one K=3T block-diag-ones matmul
folds the ring into y[T, chunk]; sigmoid+b3; DMA out.
"""
import sys
import os

_REPO = '/opt/trn_rl_repo'
if _REPO not in sys.path:
    sys.path.insert(0, _REPO)

import numpy as np  # noqa: E402

T, H, W, F = 12, 64, 64, 64
B = 8
PW = H + 2                 # padded width = 66
FRAME = PW * PW            # 4356
FSTRIDE = FRAME + 4        # frame stride per channel = 4360
NCHUNK = 8                 # pixel chunks per frame
CROWS = H // NCHUNK        # 8 rows per chunk
CHUNK = CROWS * W          # 512 pixels per chunk
INTER = PW + 1             # interior base offset = 67
XLEAD = PW + 1             # lead zeros in x_im buffer
XLEN = XLEAD + T * FRAME + 160

# tap (dy,dx) with dy,dx in 0..2 -> flat offset (dy-1)*PW + (dx-1)
PAIRS_A = [((0, 0), (0, 1)), ((1, 0), (1, 1)), ((2, 0), (2, 1))]
PAIR_B = ((0, 2), (1, 2))
SINGLE_A = (2, 2)
REC_SINGLES = [(0, 2), (1, 2), (2, 2)]


def _off(t):
    return (t[0] - 1) * PW + (t[1] - 1)


A_PAIR_BASE = [INTER + _off(p[0]) for p in PAIRS_A]          # 0, 66, 132
B_PAIR_BASE = INTER + _off(PAIR_B[0])                        # 2
A_SINGLE_BASE = INTER + _off(SINGLE_A)                       # 134
REC_SINGLE_BASE = [INTER + _off(t) for t in REC_SINGLES]     # 2, 68, 134


def pack_weights(ks, rks):
    """Pack conv weights into [NT,128,128] f32 array + per-layer group lists.

    groups[layer] = {'inp': [...], 'rec': [...]}, entries
    (tile_m0, tile_m1, k_parts, in_tile, base_off); in_tile 0=A, 1=B.
    """
    tiles = []
    groups = []

    def add_pair(k, ta, tb, m, swap=False):
        # moving tile halves: lower supplies ta, upper tb (swap=False: input
        # frames, data@lower / shift+1@upper). For h_cur (data@upper,
        # shift+1@lower) the lower half reads h@+1 = tb, upper h = ta.
        w = np.zeros((128, 128), np.float32)
        lo, hi = (tb, ta) if swap else (ta, tb)
        w[0:64, :] = k[lo[0], lo[1], :, m * 128:(m + 1) * 128]
        w[64:128, :] = k[hi[0], hi[1], :, m * 128:(m + 1) * 128]
        tiles.append(w)
        return len(tiles) - 1

    def add_single(k, t, m, upper=False):
        w = np.zeros((128, 128), np.float32)
        r0 = 64 if upper else 0
        w[r0:r0 + 64, :] = k[t[0], t[1], :, m * 128:(m + 1) * 128]
        tiles.append(w)
        return len(tiles) - 1

    for layer in range(3):
        k, rk = ks[layer], rks[layer]
        g = {'inp': [], 'rec': []}
        if layer == 0:
            k9 = k.reshape(9, 4 * F)
            for m in range(2):
                w = np.zeros((128, 128), np.float32)
                w[0:9, :] = k9[:, m * 128:(m + 1) * 128]
                tiles.append(w)
            g['inp'].append((len(tiles) - 2, len(tiles) - 1, 9, 0, 0))
        else:
            for i, (ta, tb) in enumerate(PAIRS_A):
                g['inp'].append((add_pair(k, ta, tb, 0),
                                 add_pair(k, ta, tb, 1),
                                 128, 0, A_PAIR_BASE[i]))
            g['inp'].append((add_pair(k, PAIR_B[0], PAIR_B[1], 0),
                             add_pair(k, PAIR_B[0], PAIR_B[1], 1),
                             128, 1, B_PAIR_BASE))
            g['inp'].append((add_single(k, SINGLE_A, 0),
                             add_single(k, SINGLE_A, 1),
                             64, 0, A_SINGLE_BASE))
        for i, (ta, tb) in enumerate(PAIRS_A):
            g['rec'].append((add_pair(rk, ta, tb, 0, swap=True),
                             add_pair(rk, ta, tb, 1, swap=True),
                             128, 0, A_PAIR_BASE[i]))
        for i, t in enumerate(REC_SINGLES):
            g['rec'].append((add_single(rk, t, 0, upper=True),
                             add_single(rk, t, 1, upper=True),
                             -64, 0, REC_SINGLE_BASE[i]))
        groups.append(g)

    return np.stack(tiles), groups


N_WTILES = 58  # (2 + 12) L0 + (10 + 12) * 2


def pack_w3(w3):
    """conv3d weights -> [5,128,3] tiles + table (tile, k_parts, in_tile, base)."""
    w3 = w3[:, :, :, :, 0]  # [3(dt), 3, 3, 64]
    tiles = np.zeros((5, 128, 3), np.float32)
    table = []
    for i, (ta, tb) in enumerate(PAIRS_A):
        for m in range(3):
            tiles[i, 0:64, m] = w3[m, ta[0], ta[1], :]
            tiles[i, 64:128, m] = w3[m, tb[0], tb[1], :]
        table.append((i, 128, 0, A_PAIR_BASE[i]))
    for m in range(3):
        tiles[3, 0:64, m] = w3[m, PAIR_B[0][0], PAIR_B[0][1], :]
        tiles[3, 64:128, m] = w3[m, PAIR_B[1][0], PAIR_B[1][1], :]
    table.append((3, 128, 1, B_PAIR_BASE))
    for m in range(3):
        tiles[4, 0:64, m] = w3[m, SINGLE_A[0], SINGLE_A[1], :]
    table.append((4, 64, 0, A_SINGLE_BASE))
    return tiles, table


def build_nc(TT=T):
    import concourse.bass as bass
    import concourse.mybir as mybir
    import concourse.tile as tile
    from concourse import bacc

    F32, F32R = mybir.dt.float32, mybir.dt.float32r
    AF = mybir.ActivationFunctionType

    nc = bacc.Bacc("TRN2", target_bir_lowering=False, debug=False,
                   num_devices=8)

    d_x = nc.dram_tensor("x_im", [1, XLEN], F32R, kind="ExternalInput")
    d_wt = nc.dram_tensor("wt", [N_WTILES, 128, 128], F32R,
                          kind="ExternalInput")
    d_w3 = nc.dram_tensor("w3t", [5, 128, 3], F32R, kind="ExternalInput")
    d_bd = nc.dram_tensor("bd", [3 * TT, TT], F32R, kind="ExternalInput")
    d_b = nc.dram_tensor("b_all", [3, 256], F32, kind="ExternalInput")
    d_gb = nc.dram_tensor("gb_all", [3, 64], F32, kind="ExternalInput")
    d_be = nc.dram_tensor("be_all", [3, 64], F32, kind="ExternalInput")
    d_b3 = nc.dram_tensor("b3b", [TT, 1], F32, kind="ExternalInput")
    d_y = nc.dram_tensor("y", [TT, H * W], F32, kind="ExternalOutput")
    DBG = bool(os.environ.get("KDBG"))
    d_dbg = [nc.dram_tensor(f"dbg{l}", [TT, 64, FRAME], F32,
                            kind="ExternalOutput") if DBG else None
             for l in range(3)]

    _, wgroups = pack_weights(
        [np.zeros((3, 3, 1, 256), np.float32)] +
        [np.zeros((3, 3, 64, 256), np.float32)] * 2,
        [np.zeros((3, 3, 64, 256), np.float32)] * 3)
    _, w3table = pack_w3(np.zeros((3, 3, 3, 64, 1), np.float32))

    def sub_ap(tile_obj, p0, np_, free_off, free_dims):
        base = tile_obj[:]
        ps = base.ap[0][0]
        return bass.AP(base.tensor, base.offset + p0 * ps + free_off,
                       [[ps, np_]] + [list(d) for d in free_dims])

    def conv_rhs(tile_obj, kparts, base, chunk):
        return sub_ap(tile_obj, 0, kparts, base + chunk * CROWS * PW,
                      [[PW, CROWS], [1, W]])

    def interior_ap(tile_obj, chunk, p0=64):
        return sub_ap(tile_obj, p0, 64, INTER + chunk * CROWS * PW,
                      [[PW, CROWS], [1, W]])

    with tile.TileContext(nc) as tc:
        with tc.tile_pool(name="small", bufs=1) as small, \
             tc.tile_pool(name="state", bufs=1) as state, \
             tc.tile_pool(name="gates", bufs=2) as gates, \
             tc.tile_pool(name="dram", bufs=1, space="DRAM") as dpool:

            # ---- biases / bn params ----
            b_if, b_go, gb_t, be_t = [], [], [], []
            for l in range(3):
                tt1 = small.tile([128, 1], F32, tag=f"bif{l}")
                nc.sync.dma_start(
                    tt1[:], d_b[:][l, 0:128].rearrange("(a o) -> a o", o=1))
                b_if.append(tt1)
                tt2 = small.tile([128, 1], F32, tag=f"bgo{l}")
                nc.sync.dma_start(
                    tt2[:], d_b[:][l, 128:256].rearrange("(a o) -> a o", o=1))
                b_go.append(tt2)
                tt4 = small.tile([128, 1], F32, tag=f"gb{l}")
                nc.sync.dma_start(
                    tt4[64:128, :],
                    d_gb[:][l, :].rearrange("(a o) -> a o", o=1))
                gb_t.append(tt4)
                tt5 = small.tile([128, 1], F32, tag=f"be{l}")
                nc.sync.dma_start(
                    tt5[64:128, :],
                    d_be[:][l, :].rearrange("(a o) -> a o", o=1))
                be_t.append(tt5)
            b3t = small.tile([TT, 1], F32, tag="b3")
            nc.sync.dma_start(b3t[:], d_b3[:])

            # ---- persistent state ----
            hcur = [state.tile([128, FSTRIDE], F32R, tag=f"hcur{i}",
                                name=f"hcur{i}") for i in range(2)]
            for hc in hcur:
                nc.vector.memset(hc[:].bitcast(F32), 0.0)
            c_t = state.tile([128, H * W], F32, tag="c")
            stage = state.tile([128, FSTRIDE], F32R, tag="stage")
            nc.vector.memset(stage[:].bitcast(F32), 0.0)

            scratch = [dpool.tile([TT, 64, FSTRIDE], F32R, tag=f"scr{i}",
                                  name=f"scr{i}") for i in range(2)]

            def frame_dma_A(dst, scr, t):
                src = scr[:][t, :, :].rearrange("a b -> (a b)")
                ap = bass.AP(src.tensor, src.offset,
                             [[1, 2], [FSTRIDE, 64], [1, FRAME]])
                nc.sync.dma_start(dst[:, 0:FRAME], ap)

            def frame_dma_B(dst, scr, t):
                src = scr[:][t, :, :].rearrange("a b -> (a b)")
                ap = bass.AP(src.tensor, src.offset,
                             [[PW, 2], [FSTRIDE, 64], [1, FRAME - PW]])
                nc.sync.dma_start(dst[:, 0:FRAME - PW], ap)

            # ================= ConvLSTM layers =================
            for layer in range(3):
                g = wgroups[layer]
                rd_scr = scratch[(layer + 1) % 2]
                wr_scr = scratch[layer % 2]
                wneed = sorted({i for e in g['inp'] + g['rec']
                                for i in (e[0], e[1])})
                with tc.tile_pool(name=f"w{layer}", bufs=1) as wp, \
                     tc.tile_pool(name=f"fr{layer}", bufs=2) as frp, \
                     tc.tile_pool(name=f"ps{layer}", bufs=3,
                                  space="PSUM") as psp:
                    wtiles = {}
                    for i in wneed:
                        wti = wp.tile([128, 128], F32R, tag=f"wt{i}")
                        nc.sync.dma_start(wti[:], d_wt[:][i, :, :])
                        wtiles[i] = wti

                    for t in range(TT):
                        hprev = hcur[(t + 1) % 2]
                        hnew = hcur[t % 2]

                        if layer == 0:
                            imt = frp.tile([9, FSTRIDE], F32R, tag="im")
                            xap = d_x[:].rearrange("o n -> (o n)")
                            src = bass.AP(xap.tensor, xap.offset + XLEAD + t * FRAME,
                                          [[PW, 3], [1, 3], [1, FSTRIDE]])
                            nc.sync.dma_start(imt[:], src)
                            inA = inB = None
                        else:
                            inA = frp.tile([128, FSTRIDE], F32R, tag="inA")
                            frame_dma_A(inA, rd_scr, t)
                            inB = frp.tile([128, FSTRIDE], F32R, tag="inB")
                            frame_dma_B(inB, rd_scr, t)

                        for chunk in range(NCHUNK):
                            psA = psp.tile([128, CHUNK], F32, tag="psA")
                            psB = psp.tile([128, CHUNK], F32, tag="psB")
                            for m, pst in ((0, psA), (1, psB)):
                                mms = []
                                for (i0, i1, kp, itile, base) in g['inp']:
                                    wi = wtiles[i0 if m == 0 else i1]
                                    if layer == 0:
                                        rhs = conv_rhs(imt, 9, 0, chunk)
                                        mms.append((wi[0:9, :], rhs))
                                    else:
                                        st = inA if itile == 0 else inB
                                        rhs = conv_rhs(st, kp, base, chunk)
                                        mms.append((wi[0:kp, :], rhs))
                                if t > 0:
                                    for (i0, i1, kp, itile, base) in g['rec']:
                                        wi = wtiles[i0 if m == 0 else i1]
                                        if kp == -64:
                                            rhs = sub_ap(
                                                hprev, 64, 64,
                                                base + chunk * CROWS * PW,
                                                [[PW, CROWS], [1, W]])
                                            mms.append((wi[64:128, :], rhs))
                                        else:
                                            rhs = conv_rhs(hprev, kp, base,
                                                           chunk)
                                            mms.append((wi[0:kp, :], rhs))
                                nmm = len(mms)
                                for j, (lw, rhs) in enumerate(mms):
                                    nc.tensor.matmul(pst[:], lw, rhs,
                                                     start=(j == 0),
                                                     stop=(j == nmm - 1))

                            # i,f in psA; g,o in psB. c/h live on
                            # partitions 64:128 (same as f,o); only ig is
                            # computed at base 0 and re-based via DMA.
                            nc.scalar.activation(psA[:], psA[:], AF.Sigmoid,
                                                 bias=b_if[layer][:])
                            g_t = gates.tile([64, CHUNK], F32, tag="g_t")
                            nc.scalar.activation(g_t[:], psB[0:64, :],
                                                 AF.Tanh,
                                                 bias=b_go[layer][0:64, :])
                            nc.scalar.activation(psB[64:128, :],
                                                 psB[64:128, :], AF.Sigmoid,
                                                 bias=b_go[layer][64:128, :])
                            ig = gates.tile([64, CHUNK], F32, tag="ig")
                            nc.vector.tensor_mul(ig[:], psA[0:64, :], g_t[:])
                            ig64 = gates.tile([128, CHUNK], F32, tag="ig64")
                            nc.sync.dma_start(ig64[64:128, :], ig[:])
                            csl = c_t[64:128,
                                      chunk * CHUNK:(chunk + 1) * CHUNK]
                            if t == 0:
                                nc.vector.tensor_copy(csl, ig64[64:128, :])
                            else:
                                nc.vector.tensor_mul(csl, csl,
                                                     psA[64:128, :])
                                nc.vector.tensor_add(csl, csl,
                                                     ig64[64:128, :])
                            tc128 = gates.tile([128, CHUNK], F32, tag="tc")
                            nc.scalar.activation(tc128[64:128, :], csl,
                                                 AF.Tanh)
                            nc.vector.tensor_mul(interior_ap(hnew, chunk),
                                                 psB[64:128, :],
                                                 tc128[64:128, :])
                            nc.scalar.activation(interior_ap(stage, chunk),
                                                 interior_ap(hnew, chunk),
                                                 AF.Relu,
                                                 bias=be_t[layer][64:128, :],
                                                 scale=gb_t[layer][64:128, :])

                        nc.sync.dma_start(
                            sub_ap(hnew, 0, 64, 0, [[1, FRAME]]),
                            sub_ap(hnew, 64, 64, 1, [[1, FRAME]]))
                        nc.sync.dma_start(wr_scr[:][t, :, 0:FRAME],
                                          stage[64:128, 0:FRAME])
                        if DBG:
                            nc.sync.dma_start(
                                d_dbg[layer][:][t, :, :].bitcast(F32R),
                                stage[64:128, 0:FRAME])

            # ================= Conv3d + sigmoid =================
            cv_scr = scratch[2 % 2]
            with tc.tile_pool(name="w3p", bufs=1) as w3p, \
                 tc.tile_pool(name="frc", bufs=2) as frc, \
                 tc.tile_pool(name="c3", bufs=2) as c3p, \
                 tc.tile_pool(name="ringp", bufs=1) as ringp, \
                 tc.tile_pool(name="psc", bufs=3, space="PSUM") as pscp:
                w3tiles = []
                for i in range(5):
                    w3i = w3p.tile([128, 3], F32R, tag=f"w3_{i}")
                    nc.sync.dma_start(w3i[:], d_w3[:][i, :, :])
                    w3tiles.append(w3i)
                bdt = w3p.tile([3 * TT, TT], F32R, tag="bd")
                nc.sync.dma_start(bdt[:], d_bd[:])

                ring = ringp.tile([3 * TT, H * W], F32, tag="ring")
                nc.vector.memset(ring[:], 0.0)

                for tp in range(TT):
                    inA = frc.tile([128, FSTRIDE], F32R, tag="inA")
                    frame_dma_A(inA, cv_scr, tp)
                    inB = frc.tile([128, FSTRIDE], F32R, tag="inB")
                    frame_dma_B(inB, cv_scr, tp)
                    for chunk in range(NCHUNK):
                        pP = pscp.tile([3, CHUNK], F32, tag="pP")
                        n3 = len(w3table)
                        for j, (wi, kp, itile, base) in enumerate(w3table):
                            st = inA if itile == 0 else inB
                            rhs = conv_rhs(st, kp, base, chunk)
                            nc.tensor.matmul(pP[:], w3tiles[wi][0:kp, :],
                                             rhs, start=(j == 0),
                                             stop=(j == n3 - 1))
                        pstg = c3p.tile([3, CHUNK], F32, tag="pstg")
                        nc.scalar.activation(pstg[:], pP[:], AF.Copy)
                        for m in range(3):
                            tt = tp + 1 - m
                            if 0 <= tt < TT:
                                nc.sync.dma_start(
                                    ring[m * TT + tt: m * TT + tt + 1,
                                         chunk * CHUNK:(chunk + 1) * CHUNK],
                                    pstg[m:m + 1, :])
                ring_r = ringp.tile([3 * TT, H * W], F32R, tag="ring_r")
                nc.vector.tensor_copy(ring_r[:], ring[:])
                for chunk in range(NCHUNK):
                    pY = pscp.tile([TT, CHUNK], F32, tag="pY")
                    nc.tensor.matmul(
                        pY[:], bdt[:],
                        ring_r[:, chunk * CHUNK:(chunk + 1) * CHUNK],
                        start=True, stop=True)
                    ystg = c3p.tile([TT, CHUNK], F32, tag="ystg")
                    nc.scalar.activation(ystg[:], pY[:], AF.Sigmoid,
                                         bias=b3t[:])
                    nc.sync.dma_start(
                        d_y[:][:, chunk * CHUNK:(chunk + 1) * CHUNK],
                        ystg[:])

    nc.compile()
    return nc


def prep_inputs(x, k0, rk0, b0, g0, be0, k1, rk1, b1, g1, be1,
                k2, rk2, b2, g2, be2, w3, b3, TT=T):
    x = np.asarray(x, np.float32)
    wt, _ = pack_weights(
        [np.asarray(k0, np.float32), np.asarray(k1, np.float32),
         np.asarray(k2, np.float32)],
        [np.asarray(rk0, np.float32), np.asarray(rk1, np.float32),
         np.asarray(rk2, np.float32)])
    w3t, _ = pack_w3(np.asarray(w3, np.float32))
    b_all = np.stack([np.asarray(b0, np.float32),
                      np.asarray(b1, np.float32),
                      np.asarray(b2, np.float32)])
    scale = np.float32(1.0 / np.sqrt(1.0 + 1e-3))
    gb_all = np.stack([np.asarray(g0, np.float32) * scale,
                       np.asarray(g1, np.float32) * scale,
                       np.asarray(g2, np.float32) * scale])
    be_all = np.stack([np.asarray(be0, np.float32),
                       np.asarray(be1, np.float32),
                       np.asarray(be2, np.float32)])
    bd = np.zeros((3 * TT, TT), np.float32)
    for m in range(3):
        for t in range(TT):
            bd[m * TT + t, t] = 1.0
    b3b = np.full((TT, 1), np.asarray(b3, np.float32).ravel()[0], np.float32)

    shared = dict(wt=wt, w3t=w3t, bd=bd, b_all=b_all, gb_all=gb_all,
                  be_all=be_all, b3b=b3b)
    in_maps = []
    for bb in range(B):
        xi = np.zeros((1, XLEN), np.float32)
        fr = np.zeros((TT, PW, PW), np.float32)
        fr[:, 1:H + 1, 1:W + 1] = x[bb, :TT, :, :, 0]
        xi[0, XLEAD:XLEAD + TT * FRAME] = fr.reshape(-1)
        m = dict(shared)
        m["x_im"] = xi
        in_maps.append(m)
    return in_maps


_CACHED = {}


def kernel(**inputs):
    from concourse.bass_utils import run_bass_kernel_spmd
    if 'nc' not in _CACHED:
        _CACHED['nc'] = build_nc(T)
    nc = _CACHED['nc']
    in_maps = prep_inputs(**inputs)
    res = run_bass_kernel_spmd(nc, in_maps, core_ids=list(range(B)),
                               trace=bool(os.environ.get('KTRACE')))
    _CACHED['last_res'] = res
    y = np.stack([r["y"].reshape(T, H, W, 1) for r in res.results])
    return y


# revision 12
# speedup vs baseline: 1.6374x; 1.6374x over previous
"""Trainium2 Bass kernel for 3-layer ConvLSTM2D stack + BN/ReLU + Conv3D+sigmoid.

Model (per reference):
  for l in 0..2:  h = bn_relu(conv_lstm(h, k_l, rk_l, b_l), g_l, be_l)
  y = sigmoid(conv3d(h, w3) + b3)

Shapes: x [B=8, T=12, 64, 64, 1], F=64 filters per layer, 3x3 kernels, SAME.

Sharding: data-parallel over B across 8 NeuronCores (1 image/core), weights
replicated.

Layout: channels on SBUF partitions, pixels on the free axis in zero-padded
66x66 frames (flat 4356, stride 4360). A 3x3 conv = 9 shifted-tap matmuls
accumulated in PSUM (K=chans, M=out-chans, N=pixels), taps K-packed 2-per-
matmul via shifted replicas on the other 64 partitions of the moving tile:
  tile A = (src, src shifted +1); tile B = (src, src shifted +66)
Recurrent-path matmuls run in float32r (11-bit mantissa); inter-layer
activations (bn_relu outputs) round-trip through DRAM in bf16.

Gates: z lives in PSUM as psA=[i;f], psB=[g;o]. c/h reside on partitions
64:128 (same as f,o) so the c-chain is base-aligned; i*g (base 0) is
relocated to base 64 with a tiny identity matmul through PSUM. BN+ReLU is
one ScalarE op (scale=g/sqrt(1+eps), bias=be).

DMAs are spread over both HWDGE rings (nc.sync + nc.scalar).
"""
import sys
import os

_REPO = '/opt/trn_rl_repo'
if _REPO not in sys.path:
    sys.path.insert(0, _REPO)

import numpy as np  # noqa: E402
import ml_dtypes  # noqa: E402

T, H, W, F = 12, 64, 64, 64
B = 8
PW = H + 2                 # padded width = 66
FRAME = PW * PW            # 4356
FSTRIDE = FRAME + 4        # frame stride per channel = 4360
NCHUNK = 8
CROWS = H // NCHUNK        # 8 rows per chunk
CHUNK = CROWS * W          # 512 pixels per chunk
INTER = PW + 1             # interior base offset = 67
XLEAD = PW + 1
XLEN = XLEAD + T * FRAME + 160

PAIRS_A = [((0, 0), (0, 1)), ((1, 0), (1, 1)), ((2, 0), (2, 1))]
PAIR_B = ((0, 2), (1, 2))
SINGLE_A = (2, 2)
REC_SINGLES = [(0, 2), (1, 2), (2, 2)]


def _off(t):
    return (t[0] - 1) * PW + (t[1] - 1)


A_PAIR_BASE = [INTER + _off(p[0]) for p in PAIRS_A]          # 0, 66, 132
B_PAIR_BASE = INTER + _off(PAIR_B[0])                        # 2
A_SINGLE_BASE = INTER + _off(SINGLE_A)                       # 134
REC_SINGLE_BASE = [INTER + _off(t) for t in REC_SINGLES]     # 2, 68, 134

BF16 = ml_dtypes.bfloat16


def pack_weights(ks, rks):
    """Pack conv weights.

    Returns (wt_inp[bf16], wt_rec[f32], groups); groups[layer] =
    {'inp': [(i0, i1, kp, in_tile, base)], 'rec': [...]}; in_tile 0=A 1=B;
    kp: 128 pair, 64 single (lower), -64 single (upper, h_cur data half).
    """
    ti, tr = [], []
    groups = []

    def addt(lst, w):
        lst.append(w)
        return len(lst) - 1

    def pair(k, ta, tb, m, swap):
        w = np.zeros((128, 128), np.float32)
        lo, hi = (tb, ta) if swap else (ta, tb)
        w[0:64, :] = k[lo[0], lo[1], :, m * 128:(m + 1) * 128]
        w[64:128, :] = k[hi[0], hi[1], :, m * 128:(m + 1) * 128]
        return w

    def single(k, t, m, upper):
        w = np.zeros((128, 128), np.float32)
        r0 = 64 if upper else 0
        w[r0:r0 + 64, :] = k[t[0], t[1], :, m * 128:(m + 1) * 128]
        return w

    for layer in range(3):
        k, rk = ks[layer], rks[layer]
        g = {'inp': [], 'rec': []}
        if layer == 0:
            k9 = k.reshape(9, 4 * F)
            idx = []
            for m in range(2):
                w = np.zeros((128, 128), np.float32)
                w[0:9, :] = k9[:, m * 128:(m + 1) * 128]
                idx.append(addt(ti, w))
            g['inp'].append((idx[0], idx[1], 9, 0, 0))
        else:
            for i, (ta, tb) in enumerate(PAIRS_A):
                g['inp'].append((addt(ti, pair(k, ta, tb, 0, False)),
                                 addt(ti, pair(k, ta, tb, 1, False)),
                                 128, 0, A_PAIR_BASE[i]))
            g['inp'].append((addt(ti, pair(k, PAIR_B[0], PAIR_B[1], 0,
                                           False)),
                             addt(ti, pair(k, PAIR_B[0], PAIR_B[1], 1,
                                           False)),
                             128, 1, B_PAIR_BASE))
            g['inp'].append((addt(ti, single(k, SINGLE_A, 0, False)),
                             addt(ti, single(k, SINGLE_A, 1, False)),
                             64, 0, A_SINGLE_BASE))
        for i, (ta, tb) in enumerate(PAIRS_A):
            g['rec'].append((addt(tr, pair(rk, ta, tb, 0, True)),
                             addt(tr, pair(rk, ta, tb, 1, True)),
                             128, 0, A_PAIR_BASE[i]))
        for i, t in enumerate(REC_SINGLES):
            g['rec'].append((addt(tr, single(rk, t, 0, True)),
                             addt(tr, single(rk, t, 1, True)),
                             -64, 0, REC_SINGLE_BASE[i]))
        groups.append(g)

    return (np.stack(ti).astype(BF16), np.stack(tr).astype(np.float32),
            groups)


N_WTI = 12  # 2 (L0) + 10 (L1) ... actually 2 + 10 + 10 = 22
N_WTI = 22
N_WTR = 36  # 12 * 3


def pack_w3(w3):
    """conv3d weights -> bf16 [5,128,4] tiles (3 cols used) + table."""
    w3 = w3[:, :, :, :, 0]  # [3(dt), 3, 3, 64]
    tiles = np.zeros((5, 128, 4), np.float32)
    table = []
    for i, (ta, tb) in enumerate(PAIRS_A):
        for m in range(3):
            tiles[i, 0:64, m] = w3[m, ta[0], ta[1], :]
            tiles[i, 64:128, m] = w3[m, tb[0], tb[1], :]
        table.append((i, 128, 0, A_PAIR_BASE[i]))
    for m in range(3):
        tiles[3, 0:64, m] = w3[m, PAIR_B[0][0], PAIR_B[0][1], :]
        tiles[3, 64:128, m] = w3[m, PAIR_B[1][0], PAIR_B[1][1], :]
    table.append((3, 128, 1, B_PAIR_BASE))
    for m in range(3):
        tiles[4, 0:64, m] = w3[m, SINGLE_A[0], SINGLE_A[1], :]
    table.append((4, 64, 0, A_SINGLE_BASE))
    return tiles.astype(BF16), table


def build_nc(TT=T):
    import concourse.bass as bass
    import concourse.mybir as mybir
    import concourse.tile as tile
    from concourse import bacc

    F32, F32R, BF = mybir.dt.float32, mybir.dt.float32r, mybir.dt.bfloat16
    AF = mybir.ActivationFunctionType

    nc = bacc.Bacc("TRN2", target_bir_lowering=False, debug=False,
                   num_devices=8)

    d_x = nc.dram_tensor("x_im", [1, XLEN], BF, kind="ExternalInput")
    d_wti = nc.dram_tensor("wti", [N_WTI, 128, 128], BF, kind="ExternalInput")
    d_wtr = nc.dram_tensor("wtr", [N_WTR, 128, 128], F32R,
                           kind="ExternalInput")
    d_w3 = nc.dram_tensor("w3t", [5, 128, 4], BF, kind="ExternalInput")
    d_bd = nc.dram_tensor("bd", [3 * TT, TT], F32R, kind="ExternalInput")
    d_id = nc.dram_tensor("ident", [64, 128], F32R, kind="ExternalInput")
    d_b = nc.dram_tensor("b_all", [3, 256], F32, kind="ExternalInput")
    d_gb = nc.dram_tensor("gb_all", [3, 64], F32, kind="ExternalInput")
    d_be = nc.dram_tensor("be_all", [3, 64], F32, kind="ExternalInput")
    d_b3 = nc.dram_tensor("b3b", [TT, 1], F32, kind="ExternalInput")
    d_y = nc.dram_tensor("y", [TT, H * W], F32, kind="ExternalOutput")
    DBG = bool(os.environ.get("KDBG"))
    d_dbg = [nc.dram_tensor(f"dbg{l}", [TT, 64, FRAME], BF,
                            kind="ExternalOutput") if DBG else None
             for l in range(3)]

    _, _, wgroups = pack_weights(
        [np.zeros((3, 3, 1, 256), np.float32)] +
        [np.zeros((3, 3, 64, 256), np.float32)] * 2,
        [np.zeros((3, 3, 64, 256), np.float32)] * 3)
    _, w3table = pack_w3(np.zeros((3, 3, 3, 64, 1), np.float32))

    def sub_ap(tile_obj, p0, np_, free_off, free_dims):
        base = tile_obj[:]
        ps = base.ap[0][0]
        return bass.AP(base.tensor, base.offset + p0 * ps + free_off,
                       [[ps, np_]] + [list(d) for d in free_dims])

    def conv_rhs(tile_obj, kparts, base, chunk):
        return sub_ap(tile_obj, 0, kparts, base + chunk * CROWS * PW,
                      [[PW, CROWS], [1, W]])

    def interior_ap(tile_obj, chunk, p0=64):
        return sub_ap(tile_obj, p0, 64, INTER + chunk * CROWS * PW,
                      [[PW, CROWS], [1, W]])

    with tile.TileContext(nc) as tc:
        with tc.tile_pool(name="small", bufs=1) as small, \
             tc.tile_pool(name="gates", bufs=3) as gates, \
             tc.tile_pool(name="dram", bufs=1, space="DRAM") as dpool:

            b_if, b_go, gb_t, be_t = [], [], [], []
            for l in range(3):
                tt1 = small.tile([128, 1], F32, tag=f"bif{l}")
                nc.sync.dma_start(
                    tt1[:], d_b[:][l, 0:128].rearrange("(a o) -> a o", o=1))
                b_if.append(tt1)
                tt2 = small.tile([128, 1], F32, tag=f"bgo{l}")
                nc.sync.dma_start(
                    tt2[:], d_b[:][l, 128:256].rearrange("(a o) -> a o", o=1))
                b_go.append(tt2)
                tt4 = small.tile([128, 1], F32, tag=f"gb{l}")
                nc.sync.dma_start(
                    tt4[64:128, :],
                    d_gb[:][l, :].rearrange("(a o) -> a o", o=1))
                gb_t.append(tt4)
                tt5 = small.tile([128, 1], F32, tag=f"be{l}")
                nc.sync.dma_start(
                    tt5[64:128, :],
                    d_be[:][l, :].rearrange("(a o) -> a o", o=1))
                be_t.append(tt5)
            b3t = small.tile([TT, 1], F32, tag="b3")
            nc.sync.dma_start(b3t[:], d_b3[:])
            ident = small.tile([64, 128], F32R, tag="ident")
            nc.sync.dma_start(ident[:], d_id[:])

            scratch = [dpool.tile([TT, 64, FSTRIDE], BF, tag=f"scr{i}",
                                  name=f"scr{i}") for i in range(2)]

            def frame_dma_A(dst, scr, t):
                src = scr[:][t, :, :].rearrange("a b -> (a b)")
                ap = bass.AP(src.tensor, src.offset,
                             [[1, 2], [FSTRIDE, 64], [1, FRAME]])
                nc.scalar.dma_start(dst[:, 0:FRAME], ap)

            def frame_dma_B(dst, scr, t):
                src = scr[:][t, :, :].rearrange("a b -> (a b)")
                ap = bass.AP(src.tensor, src.offset,
                             [[PW, 2], [FSTRIDE, 64], [1, FRAME - PW]])
                nc.scalar.dma_start(dst[:, 0:FRAME - PW], ap)

            # ================= ConvLSTM layers =================
            with tc.tile_pool(name="state", bufs=1) as state:
                hcur = [state.tile([128, FSTRIDE], F32R, tag=f"hcur{i}",
                                   name=f"hcur{i}") for i in range(2)]
                for hc in hcur:
                    nc.vector.memset(hc[:].bitcast(F32), 0.0)
                c_t = state.tile([128, H * W], F32, tag="c")
                stage = state.tile([128, FSTRIDE], BF, tag="stage")
                nc.vector.memset(stage[:], 0.0)

                for layer in range(3):
                    g = wgroups[layer]
                    rd_scr = scratch[(layer + 1) % 2]
                    wr_scr = scratch[layer % 2]
                    with tc.tile_pool(name=f"w{layer}", bufs=1) as wp, \
                         tc.tile_pool(name=f"fr{layer}", bufs=3) as frp, \
                         tc.tile_pool(name=f"ps{layer}", bufs=3,
                                      space="PSUM") as psp, \
                         tc.tile_pool(name=f"psig{layer}", bufs=2,
                                      space="PSUM") as psig:
                        wti_t, wtr_t = {}, {}
                        for e in g['inp']:
                            for i in (e[0], e[1]):
                                if i not in wti_t:
                                    wt_ = wp.tile([128, 128], BF,
                                                  tag=f"wti{i}",
                                                  name=f"wti{i}")
                                    nc.sync.dma_start(wt_[:],
                                                      d_wti[:][i, :, :])
                                    wti_t[i] = wt_
                        for e in g['rec']:
                            for i in (e[0], e[1]):
                                if i not in wtr_t:
                                    wt_ = wp.tile([128, 128], F32R,
                                                  tag=f"wtr{i}",
                                                  name=f"wtr{i}")
                                    nc.sync.dma_start(wt_[:],
                                                      d_wtr[:][i, :, :])
                                    wtr_t[i] = wt_

                        for t in range(TT):
                            hprev = hcur[(t + 1) % 2]
                            hnew = hcur[t % 2]

                            if layer == 0:
                                imt = frp.tile([9, FSTRIDE], BF, tag="im")
                                xap = d_x[:].rearrange("o n -> (o n)")
                                src = bass.AP(xap.tensor,
                                              xap.offset + XLEAD + t * FRAME,
                                              [[PW, 3], [1, 3], [1, FSTRIDE]])
                                nc.scalar.dma_start(imt[:], src)
                                inA = inB = None
                            else:
                                inA = frp.tile([128, FSTRIDE], BF, tag="inA")
                                frame_dma_A(inA, rd_scr, t)
                                inB = frp.tile([128, FSTRIDE], BF, tag="inB")
                                frame_dma_B(inB, rd_scr, t)

                            for chunk in range(NCHUNK):
                                psA = psp.tile([128, CHUNK], F32, tag="psA")
                                psB = psp.tile([128, CHUNK], F32, tag="psB")
                                for m, pst in ((0, psA), (1, psB)):
                                    mms = []
                                    for (i0, i1, kp, itl, base) in g['inp']:
                                        wi = wti_t[i0 if m == 0 else i1]
                                        if layer == 0:
                                            rhs = conv_rhs(imt, 9, 0, chunk)
                                            mms.append((wi[0:9, :], rhs))
                                        else:
                                            st = inA if itl == 0 else inB
                                            rhs = conv_rhs(st, kp, base,
                                                           chunk)
                                            mms.append((wi[0:kp, :], rhs))
                                    if t > 0:
                                        for (i0, i1, kp, itl, base) in \
                                                g['rec']:
                                            wi = wtr_t[i0 if m == 0 else i1]
                                            if kp == -64:
                                                rhs = sub_ap(
                                                    hprev, 64, 64,
                                                    base + chunk * CROWS * PW,
                                                    [[PW, CROWS], [1, W]])
                                                mms.append((wi[64:128, :],
                                                            rhs))
                                            else:
                                                rhs = conv_rhs(hprev, kp,
                                                               base, chunk)
                                                mms.append((wi[0:kp, :], rhs))
                                    nmm = len(mms)
                                    for j, (lw, rhs) in enumerate(mms):
                                        nc.tensor.matmul(
                                            pst[:], lw, rhs,
                                            start=(j == 0),
                                            stop=(j == nmm - 1))

                                # gates
                                nc.scalar.activation(psA[:], psA[:],
                                                     AF.Sigmoid,
                                                     bias=b_if[layer][:])
                                g_t = gates.tile([64, CHUNK], F32, tag="g_t")
                                nc.scalar.activation(
                                    g_t[:], psB[0:64, :], AF.Tanh,
                                    bias=b_go[layer][0:64, :])
                                nc.scalar.activation(
                                    psB[64:128, :], psB[64:128, :],
                                    AF.Sigmoid, bias=b_go[layer][64:128, :])
                                ig = gates.tile([64, CHUNK], F32R, tag="ig")
                                nc.vector.tensor_mul(ig[:], psA[0:64, :],
                                                     g_t[:])
                                igp = psig.tile([128, CHUNK], F32, tag="igp")
                                nc.tensor.matmul(igp[:], ident[:], ig[:],
                                                 start=True, stop=True)
                                csl = c_t[64:128,
                                          chunk * CHUNK:(chunk + 1) * CHUNK]
                                if t == 0:
                                    nc.vector.tensor_copy(csl,
                                                          igp[64:128, :])
                                else:
                                    nc.vector.tensor_mul(csl, csl,
                                                         psA[64:128, :])
                                    nc.vector.tensor_add(csl, csl,
                                                         igp[64:128, :])
                                tc128 = gates.tile([128, CHUNK], F32,
                                                   tag="tc")
                                nc.scalar.activation(tc128[64:128, :], csl,
                                                     AF.Tanh)
                                nc.vector.tensor_mul(
                                    interior_ap(hnew, chunk),
                                    psB[64:128, :], tc128[64:128, :])
                                nc.scalar.activation(
                                    interior_ap(stage, chunk),
                                    interior_ap(hnew, chunk), AF.Relu,
                                    bias=be_t[layer][64:128, :],
                                    scale=gb_t[layer][64:128, :])

                            # replica: hnew lower <- upper shifted +1
                            nc.sync.dma_start(
                                sub_ap(hnew, 0, 64, 0, [[1, FRAME]]),
                                sub_ap(hnew, 64, 64, 1, [[1, FRAME]]))
                            nc.sync.dma_start(wr_scr[:][t, :, 0:FRAME],
                                              stage[64:128, 0:FRAME])
                            if DBG:
                                nc.sync.dma_start(d_dbg[layer][:][t, :, :],
                                                  stage[64:128, 0:FRAME])

            # ================= Conv3d + sigmoid =================
            cv_scr = scratch[0]
            with tc.tile_pool(name="w3p", bufs=1) as w3p, \
                 tc.tile_pool(name="frc", bufs=3) as frc, \
                 tc.tile_pool(name="c3", bufs=2) as c3p, \
                 tc.tile_pool(name="ringp", bufs=1) as ringp, \
                 tc.tile_pool(name="psc", bufs=3, space="PSUM") as pscp:
                w3tiles = []
                for i in range(5):
                    w3i = w3p.tile([128, 4], BF, tag=f"w3_{i}")
                    nc.sync.dma_start(w3i[:], d_w3[:][i, :, :])
                    w3tiles.append(w3i)
                bdt = w3p.tile([3 * TT, TT], F32R, tag="bd")
                nc.sync.dma_start(bdt[:], d_bd[:])

                ring = ringp.tile([3 * TT, H * W], F32, tag="ring")
                nc.vector.memset(ring[:], 0.0)

                for tp in range(TT):
                    inA = frc.tile([128, FSTRIDE], BF, tag="inA")
                    frame_dma_A(inA, cv_scr, tp)
                    inB = frc.tile([128, FSTRIDE], BF, tag="inB")
                    frame_dma_B(inB, cv_scr, tp)
                    for chunk in range(NCHUNK):
                        pP = pscp.tile([3, CHUNK], F32, tag="pP")
                        n3 = len(w3table)
                        for j, (wi, kp, itl, base) in enumerate(w3table):
                            st = inA if itl == 0 else inB
                            rhs = conv_rhs(st, kp, base, chunk)
                            nc.tensor.matmul(pP[:], w3tiles[wi][0:kp, 0:3],
                                             rhs, start=(j == 0),
                                             stop=(j == n3 - 1))
                        pstg = c3p.tile([3, CHUNK], F32, tag="pstg")
                        nc.scalar.activation(pstg[:], pP[:], AF.Copy)
                        for m in range(3):
                            tt = tp + 1 - m
                            if 0 <= tt < TT:
                                nc.sync.dma_start(
                                    ring[m * TT + tt: m * TT + tt + 1,
                                         chunk * CHUNK:(chunk + 1) * CHUNK],
                                    pstg[m:m + 1, :])
                ring_r = ringp.tile([3 * TT, H * W], F32R, tag="ring_r")
                nc.vector.tensor_copy(ring_r[:], ring[:])
                for chunk in range(NCHUNK):
                    pY = pscp.tile([TT, CHUNK], F32, tag="pY")
                    nc.tensor.matmul(
                        pY[:], bdt[:],
                        ring_r[:, chunk * CHUNK:(chunk + 1) * CHUNK],
                        start=True, stop=True)
                    ystg = c3p.tile([TT, CHUNK], F32, tag="ystg")
                    nc.scalar.activation(ystg[:], pY[:], AF.Sigmoid,
                                         bias=b3t[:])
                    nc.sync.dma_start(
                        d_y[:][:, chunk * CHUNK:(chunk + 1) * CHUNK],
                        ystg[:])

    nc.compile()
    return nc


def prep_inputs(x, k0, rk0, b0, g0, be0, k1, rk1, b1, g1, be1,
                k2, rk2, b2, g2, be2, w3, b3, TT=T):
    x = np.asarray(x, np.float32)
    wti, wtr, _ = pack_weights(
        [np.asarray(k0, np.float32), np.asarray(k1, np.float32),
         np.asarray(k2, np.float32)],
        [np.asarray(rk0, np.float32), np.asarray(rk1, np.float32),
         np.asarray(rk2, np.float32)])
    w3t, _ = pack_w3(np.asarray(w3, np.float32))
    b_all = np.stack([np.asarray(b0, np.float32),
                      np.asarray(b1, np.float32),
                      np.asarray(b2, np.float32)])
    scale = np.float32(1.0 / np.sqrt(1.0 + 1e-3))
    gb_all = np.stack([np.asarray(g0, np.float32) * scale,
                       np.asarray(g1, np.float32) * scale,
                       np.asarray(g2, np.float32) * scale])
    be_all = np.stack([np.asarray(be0, np.float32),
                       np.asarray(be1, np.float32),
                       np.asarray(be2, np.float32)])
    bd = np.zeros((3 * TT, TT), np.float32)
    for m in range(3):
        for t in range(TT):
            bd[m * TT + t, t] = 1.0
    b3b = np.full((TT, 1), np.asarray(b3, np.float32).ravel()[0], np.float32)
    ident = np.concatenate([np.zeros((64, 64), np.float32),
                            np.eye(64, dtype=np.float32)], axis=1)

    shared = dict(wti=wti, wtr=wtr, w3t=w3t, bd=bd, ident=ident,
                  b_all=b_all, gb_all=gb_all, be_all=be_all, b3b=b3b)
    in_maps = []
    for bb in range(B):
        xi = np.zeros((1, XLEN), BF16)
        fr = np.zeros((TT, PW, PW), np.float32)
        fr[:, 1:H + 1, 1:W + 1] = x[bb, :TT, :, :, 0]
        xi[0, XLEAD:XLEAD + TT * FRAME] = fr.reshape(-1).astype(BF16)
        m = dict(shared)
        m["x_im"] = xi
        in_maps.append(m)
    return in_maps


_CACHED = {}


def kernel(**inputs):
    from concourse.bass_utils import run_bass_kernel_spmd
    if 'nc' not in _CACHED:
        _CACHED['nc'] = build_nc(T)
    nc = _CACHED['nc']
    in_maps = prep_inputs(**inputs)
    res = run_bass_kernel_spmd(nc, in_maps, core_ids=list(range(B)),
                               trace=bool(os.environ.get('KTRACE')))
    _CACHED['last_res'] = res
    y = np.stack([r["y"].reshape(T, H, W, 1) for r in res.results])
    return y


# revision 13
# speedup vs baseline: 1.7504x; 1.0690x over previous
"""Trainium2 Bass kernel for 3-layer ConvLSTM2D stack + BN/ReLU + Conv3D+sigmoid.

Model (per reference):
  for l in 0..2:  h = bn_relu(conv_lstm(h, k_l, rk_l, b_l), g_l, be_l)
  y = sigmoid(conv3d(h, w3) + b3)

Shapes: x [B=8, T=12, 64, 64, 1], F=64 filters per layer, 3x3 kernels, SAME.

Sharding: data-parallel over B across 8 NeuronCores (1 image/core), weights
replicated.

Layout: channels on SBUF partitions, pixels on the free axis in zero-padded
66x66 frames (flat 4356, stride 4360). A 3x3 conv = 9 shifted-tap matmuls
accumulated in PSUM (K=chans, M=out-chans, N=pixels), taps K-packed 2-per-
matmul via shifted replicas on the other 64 partitions of the moving tile:
  tile A = (src, src shifted +1); tile B = (src, src shifted +66)
Recurrent-path matmuls run in float32r (11-bit mantissa); inter-layer
activations (bn_relu outputs) round-trip through DRAM in bf16.

Gates: z lives in PSUM as psA=[i;f], psB=[g;o]. c/h reside on partitions
64:128 (same as f,o) so the c-chain is base-aligned; i*g (base 0) is
relocated to base 64 with a tiny identity matmul through PSUM. BN+ReLU is
one ScalarE op (scale=g/sqrt(1+eps), bias=be).

DMAs are spread over both HWDGE rings (nc.sync + nc.scalar).
"""
import sys
import os

_REPO = '/opt/trn_rl_repo'
if _REPO not in sys.path:
    sys.path.insert(0, _REPO)

import numpy as np  # noqa: E402
import ml_dtypes  # noqa: E402

T, H, W, F = 12, 64, 64, 64
B = 8
PW = H + 2                 # padded width = 66
FRAME = PW * PW            # 4356
FSTRIDE = FRAME + 4        # frame stride per channel = 4360
NCHUNK = 8
CROWS = H // NCHUNK        # 8 rows per chunk
CHUNK = CROWS * W          # 512 pixels per chunk
INTER = PW + 1             # interior base offset = 67
XLEAD = PW + 1
XLEN = XLEAD + T * FRAME + 160

PAIRS_A = [((0, 0), (0, 1)), ((1, 0), (1, 1)), ((2, 0), (2, 1))]
PAIR_B = ((0, 2), (1, 2))
SINGLE_A = (2, 2)
REC_SINGLES = [(0, 2), (1, 2), (2, 2)]


def _off(t):
    return (t[0] - 1) * PW + (t[1] - 1)


A_PAIR_BASE = [INTER + _off(p[0]) for p in PAIRS_A]          # 0, 66, 132
B_PAIR_BASE = INTER + _off(PAIR_B[0])                        # 2
A_SINGLE_BASE = INTER + _off(SINGLE_A)                       # 134
REC_SINGLE_BASE = [INTER + _off(t) for t in REC_SINGLES]     # 2, 68, 134

BF16 = ml_dtypes.bfloat16


def pack_weights(ks, rks):
    """Pack conv weights.

    Returns (wt_inp[bf16], wt_rec[f32], groups); groups[layer] =
    {'inp': [(i0, i1, kp, in_tile, base)], 'rec': [...]}; in_tile 0=A 1=B;
    kp: 128 pair, 64 single (lower), -64 single (upper, h_cur data half).
    """
    ti, tr = [], []
    groups = []

    def addt(lst, w):
        lst.append(w)
        return len(lst) - 1

    def pair(k, ta, tb, m, swap):
        w = np.zeros((128, 128), np.float32)
        lo, hi = (tb, ta) if swap else (ta, tb)
        w[0:64, :] = k[lo[0], lo[1], :, m * 128:(m + 1) * 128]
        w[64:128, :] = k[hi[0], hi[1], :, m * 128:(m + 1) * 128]
        return w

    def single(k, t, m, upper):
        w = np.zeros((128, 128), np.float32)
        r0 = 64 if upper else 0
        w[r0:r0 + 64, :] = k[t[0], t[1], :, m * 128:(m + 1) * 128]
        return w

    for layer in range(3):
        k, rk = ks[layer], rks[layer]
        g = {'inp': [], 'rec': []}
        if layer == 0:
            k9 = k.reshape(9, 4 * F)
            idx = []
            for m in range(2):
                w = np.zeros((128, 128), np.float32)
                w[0:9, :] = k9[:, m * 128:(m + 1) * 128]
                idx.append(addt(ti, w))
            g['inp'].append((idx[0], idx[1], 9, 0, 0))
        else:
            for i, (ta, tb) in enumerate(PAIRS_A):
                g['inp'].append((addt(ti, pair(k, ta, tb, 0, False)),
                                 addt(ti, pair(k, ta, tb, 1, False)),
                                 128, 0, A_PAIR_BASE[i]))
            g['inp'].append((addt(ti, pair(k, PAIR_B[0], PAIR_B[1], 0,
                                           False)),
                             addt(ti, pair(k, PAIR_B[0], PAIR_B[1], 1,
                                           False)),
                             128, 1, B_PAIR_BASE))
            g['inp'].append((addt(ti, single(k, SINGLE_A, 0, False)),
                             addt(ti, single(k, SINGLE_A, 1, False)),
                             64, 0, A_SINGLE_BASE))
        for i, (ta, tb) in enumerate(PAIRS_A):
            g['rec'].append((addt(tr, pair(rk, ta, tb, 0, True)),
                             addt(tr, pair(rk, ta, tb, 1, True)),
                             128, 0, A_PAIR_BASE[i]))
        for i, t in enumerate(REC_SINGLES):
            g['rec'].append((addt(tr, single(rk, t, 0, True)),
                             addt(tr, single(rk, t, 1, True)),
                             -64, 0, REC_SINGLE_BASE[i]))
        groups.append(g)

    return (np.stack(ti).astype(BF16), np.stack(tr).astype(np.float32),
            groups)


N_WTI = 12  # 2 (L0) + 10 (L1) ... actually 2 + 10 + 10 = 22
N_WTI = 22
N_WTR = 36  # 12 * 3


def pack_w3(w3):
    """conv3d weights -> bf16 [5,128,4] tiles (3 cols used) + table."""
    w3 = w3[:, :, :, :, 0]  # [3(dt), 3, 3, 64]
    tiles = np.zeros((5, 128, 4), np.float32)
    table = []
    for i, (ta, tb) in enumerate(PAIRS_A):
        for m in range(3):
            tiles[i, 0:64, m] = w3[m, ta[0], ta[1], :]
            tiles[i, 64:128, m] = w3[m, tb[0], tb[1], :]
        table.append((i, 128, 0, A_PAIR_BASE[i]))
    for m in range(3):
        tiles[3, 0:64, m] = w3[m, PAIR_B[0][0], PAIR_B[0][1], :]
        tiles[3, 64:128, m] = w3[m, PAIR_B[1][0], PAIR_B[1][1], :]
    table.append((3, 128, 1, B_PAIR_BASE))
    for m in range(3):
        tiles[4, 0:64, m] = w3[m, SINGLE_A[0], SINGLE_A[1], :]
    table.append((4, 64, 0, A_SINGLE_BASE))
    return tiles.astype(BF16), table


def build_nc(TT=T):
    import concourse.bass as bass
    import concourse.mybir as mybir
    import concourse.tile as tile
    from concourse import bacc

    F32, F32R, BF = mybir.dt.float32, mybir.dt.float32r, mybir.dt.bfloat16
    AF = mybir.ActivationFunctionType

    nc = bacc.Bacc("TRN2", target_bir_lowering=False, debug=False,
                   num_devices=8)

    d_x = nc.dram_tensor("x_im", [1, XLEN], BF, kind="ExternalInput")
    d_wti = nc.dram_tensor("wti", [N_WTI, 128, 128], BF, kind="ExternalInput")
    d_wtr = nc.dram_tensor("wtr", [N_WTR, 128, 128], F32R,
                           kind="ExternalInput")
    d_w3 = nc.dram_tensor("w3t", [5, 128, 4], BF, kind="ExternalInput")
    d_bd = nc.dram_tensor("bd", [3 * TT, TT], F32R, kind="ExternalInput")
    d_id = nc.dram_tensor("ident", [64, 128], F32R, kind="ExternalInput")
    d_b = nc.dram_tensor("b_all", [3, 256], F32, kind="ExternalInput")
    d_gb = nc.dram_tensor("gb_all", [3, 64], F32, kind="ExternalInput")
    d_be = nc.dram_tensor("be_all", [3, 64], F32, kind="ExternalInput")
    d_b3 = nc.dram_tensor("b3b", [TT, 1], F32, kind="ExternalInput")
    d_y = nc.dram_tensor("y", [TT, H * W], F32, kind="ExternalOutput")
    DBG = bool(os.environ.get("KDBG"))
    d_dbg = [nc.dram_tensor(f"dbg{l}", [TT, 64, FRAME], BF,
                            kind="ExternalOutput") if DBG else None
             for l in range(3)]

    _, _, wgroups = pack_weights(
        [np.zeros((3, 3, 1, 256), np.float32)] +
        [np.zeros((3, 3, 64, 256), np.float32)] * 2,
        [np.zeros((3, 3, 64, 256), np.float32)] * 3)
    _, w3table = pack_w3(np.zeros((3, 3, 3, 64, 1), np.float32))

    def sub_ap(tile_obj, p0, np_, free_off, free_dims):
        base = tile_obj[:]
        ps = base.ap[0][0]
        return bass.AP(base.tensor, base.offset + p0 * ps + free_off,
                       [[ps, np_]] + [list(d) for d in free_dims])

    def conv_rhs(tile_obj, kparts, base, chunk):
        return sub_ap(tile_obj, 0, kparts, base + chunk * CROWS * PW,
                      [[PW, CROWS], [1, W]])

    def interior_ap(tile_obj, chunk, p0=64):
        return sub_ap(tile_obj, p0, 64, INTER + chunk * CROWS * PW,
                      [[PW, CROWS], [1, W]])

    with tile.TileContext(nc) as tc:
        with tc.tile_pool(name="small", bufs=1) as small, \
             tc.tile_pool(name="gates", bufs=3) as gates, \
             tc.tile_pool(name="dram", bufs=1, space="DRAM") as dpool:

            b_if, b_go, gb_t, be_t = [], [], [], []
            for l in range(3):
                tt1 = small.tile([128, 1], F32, tag=f"bif{l}")
                nc.sync.dma_start(
                    tt1[:], d_b[:][l, 0:128].rearrange("(a o) -> a o", o=1))
                b_if.append(tt1)
                tt2 = small.tile([128, 1], F32, tag=f"bgo{l}")
                nc.sync.dma_start(
                    tt2[:], d_b[:][l, 128:256].rearrange("(a o) -> a o", o=1))
                b_go.append(tt2)
                tt4 = small.tile([128, 1], F32, tag=f"gb{l}")
                nc.sync.dma_start(
                    tt4[64:128, :],
                    d_gb[:][l, :].rearrange("(a o) -> a o", o=1))
                gb_t.append(tt4)
                tt5 = small.tile([128, 1], F32, tag=f"be{l}")
                nc.sync.dma_start(
                    tt5[64:128, :],
                    d_be[:][l, :].rearrange("(a o) -> a o", o=1))
                be_t.append(tt5)
            b3t = small.tile([TT, 1], F32, tag="b3")
            nc.sync.dma_start(b3t[:], d_b3[:])
            ident = small.tile([64, 128], F32R, tag="ident")
            nc.sync.dma_start(ident[:], d_id[:])

            scratch = [dpool.tile([TT, 64, FSTRIDE], BF, tag=f"scr{i}",
                                  name=f"scr{i}") for i in range(2)]

            wti_t, wtr_t = {}, {}
            for i in range(N_WTI):
                wt_ = small.tile([128, 128], BF, tag=f"wti{i}",
                                 name=f"wti{i}")
                nc.sync.dma_start(wt_[:], d_wti[:][i, :, :])
                wti_t[i] = wt_
            for i in range(N_WTR):
                wt_ = small.tile([128, 128], F32R, tag=f"wtr{i}",
                                 name=f"wtr{i}")
                nc.sync.dma_start(wt_[:], d_wtr[:][i, :, :])
                wtr_t[i] = wt_

            def frame_dma_A(dst, scr, t):
                src = scr[:][t, :, :].rearrange("a b -> (a b)")
                ap = bass.AP(src.tensor, src.offset,
                             [[1, 2], [FSTRIDE, 64], [1, FRAME]])
                nc.scalar.dma_start(dst[:, 0:FRAME], ap)

            def frame_dma_B(dst, scr, t):
                src = scr[:][t, :, :].rearrange("a b -> (a b)")
                ap = bass.AP(src.tensor, src.offset,
                             [[PW, 2], [FSTRIDE, 64], [1, FRAME - PW]])
                nc.sync.dma_start(dst[:, 0:FRAME - PW], ap)

            # ================= ConvLSTM layers =================
            with tc.tile_pool(name="state", bufs=1) as state:
                hcur = [state.tile([128, FSTRIDE], F32R, tag=f"hcur{i}",
                                   name=f"hcur{i}") for i in range(2)]
                for hc in hcur:
                    nc.vector.memset(hc[:].bitcast(F32), 0.0)
                c_t = state.tile([128, H * W], F32, tag="c")
                stage = state.tile([128, FSTRIDE], BF, tag="stage")
                nc.vector.memset(stage[:], 0.0)

                for layer in range(3):
                    g = wgroups[layer]
                    rd_scr = scratch[(layer + 1) % 2]
                    wr_scr = scratch[layer % 2]
                    with tc.tile_pool(name=f"fr{layer}", bufs=3) as frp, \
                         tc.tile_pool(name=f"ps{layer}", bufs=3,
                                      space="PSUM") as psp, \
                         tc.tile_pool(name=f"psig{layer}", bufs=2,
                                      space="PSUM") as psig:
                        for t in range(TT):
                            hprev = hcur[(t + 1) % 2]
                            hnew = hcur[t % 2]

                            if layer == 0:
                                imt = frp.tile([9, FSTRIDE], BF, tag="im")
                                xap = d_x[:].rearrange("o n -> (o n)")
                                src = bass.AP(xap.tensor,
                                              xap.offset + XLEAD + t * FRAME,
                                              [[PW, 3], [1, 3], [1, FSTRIDE]])
                                nc.scalar.dma_start(imt[:], src)
                                inA = inB = None
                            else:
                                inA = frp.tile([128, FSTRIDE], BF, tag="inA")
                                frame_dma_A(inA, rd_scr, t)
                                inB = frp.tile([128, FSTRIDE], BF, tag="inB")
                                frame_dma_B(inB, rd_scr, t)

                            for chunk in range(NCHUNK):
                                psA = psp.tile([128, CHUNK], F32, tag="psA")
                                psB = psp.tile([128, CHUNK], F32, tag="psB")
                                for m, pst in ((0, psA), (1, psB)):
                                    mms = []
                                    for (i0, i1, kp, itl, base) in g['inp']:
                                        wi = wti_t[i0 if m == 0 else i1]
                                        if layer == 0:
                                            rhs = conv_rhs(imt, 9, 0, chunk)
                                            mms.append((wi[0:9, :], rhs))
                                        else:
                                            st = inA if itl == 0 else inB
                                            rhs = conv_rhs(st, kp, base,
                                                           chunk)
                                            mms.append((wi[0:kp, :], rhs))
                                    if t > 0:
                                        for (i0, i1, kp, itl, base) in \
                                                g['rec']:
                                            wi = wtr_t[i0 if m == 0 else i1]
                                            if kp == -64:
                                                rhs = sub_ap(
                                                    hprev, 64, 64,
                                                    base + chunk * CROWS * PW,
                                                    [[PW, CROWS], [1, W]])
                                                mms.append((wi[64:128, :],
                                                            rhs))
                                            else:
                                                rhs = conv_rhs(hprev, kp,
                                                               base, chunk)
                                                mms.append((wi[0:kp, :], rhs))
                                    nmm = len(mms)
                                    for j, (lw, rhs) in enumerate(mms):
                                        nc.tensor.matmul(
                                            pst[:], lw, rhs,
                                            start=(j == 0),
                                            stop=(j == nmm - 1))

                                # gates
                                nc.scalar.activation(psA[:], psA[:],
                                                     AF.Sigmoid,
                                                     bias=b_if[layer][:])
                                g_t = gates.tile([64, CHUNK], F32, tag="g_t")
                                nc.scalar.activation(
                                    g_t[:], psB[0:64, :], AF.Tanh,
                                    bias=b_go[layer][0:64, :])
                                nc.scalar.activation(
                                    psB[64:128, :], psB[64:128, :],
                                    AF.Sigmoid, bias=b_go[layer][64:128, :])
                                ig = gates.tile([64, CHUNK], F32R, tag="ig")
                                nc.vector.tensor_mul(ig[:], psA[0:64, :],
                                                     g_t[:])
                                igp = psig.tile([128, CHUNK], F32, tag="igp")
                                nc.tensor.matmul(igp[:], ident[:], ig[:],
                                                 start=True, stop=True)
                                csl = c_t[64:128,
                                          chunk * CHUNK:(chunk + 1) * CHUNK]
                                if t == 0:
                                    nc.vector.tensor_copy(csl,
                                                          igp[64:128, :])
                                else:
                                    nc.vector.tensor_mul(csl, csl,
                                                         psA[64:128, :])
                                    nc.vector.tensor_add(csl, csl,
                                                         igp[64:128, :])
                                tc128 = gates.tile([128, CHUNK], F32,
                                                   tag="tc")
                                nc.scalar.activation(tc128[64:128, :], csl,
                                                     AF.Tanh)
                                nc.vector.tensor_mul(
                                    interior_ap(hnew, chunk),
                                    psB[64:128, :], tc128[64:128, :])
                                nc.scalar.activation(
                                    interior_ap(stage, chunk),
                                    interior_ap(hnew, chunk), AF.Relu,
                                    bias=be_t[layer][64:128, :],
                                    scale=gb_t[layer][64:128, :])

                            # replica: hnew lower <- upper shifted +1,
                            # in 4 pieces so AP-level deps release early
                            NP = 4
                            PIECE = FRAME // NP
                            for p in range(NP):
                                nc.sync.dma_start(
                                    sub_ap(hnew, 0, 64, p * PIECE,
                                           [[1, PIECE]]),
                                    sub_ap(hnew, 64, 64, p * PIECE + 1,
                                           [[1, PIECE]]))
                            nc.sync.dma_start(wr_scr[:][t, :, 0:FRAME],
                                              stage[64:128, 0:FRAME])
                            if DBG:
                                nc.sync.dma_start(d_dbg[layer][:][t, :, :],
                                                  stage[64:128, 0:FRAME])

            # ================= Conv3d + sigmoid =================
            cv_scr = scratch[0]
            with tc.tile_pool(name="w3p", bufs=1) as w3p, \
                 tc.tile_pool(name="frc", bufs=3) as frc, \
                 tc.tile_pool(name="c3", bufs=2) as c3p, \
                 tc.tile_pool(name="ringp", bufs=1) as ringp, \
                 tc.tile_pool(name="psc", bufs=3, space="PSUM") as pscp:
                w3tiles = []
                for i in range(5):
                    w3i = w3p.tile([128, 4], BF, tag=f"w3_{i}")
                    nc.sync.dma_start(w3i[:], d_w3[:][i, :, :])
                    w3tiles.append(w3i)
                bdt = w3p.tile([3 * TT, TT], F32R, tag="bd")
                nc.sync.dma_start(bdt[:], d_bd[:])

                ring = ringp.tile([3 * TT, H * W], F32, tag="ring")
                nc.vector.memset(ring[:], 0.0)

                for tp in range(TT):
                    inA = frc.tile([128, FSTRIDE], BF, tag="inA")
                    frame_dma_A(inA, cv_scr, tp)
                    inB = frc.tile([128, FSTRIDE], BF, tag="inB")
                    frame_dma_B(inB, cv_scr, tp)
                    pstf = c3p.tile([3, H * W], F32, tag="pstf")
                    for chunk in range(NCHUNK):
                        pP = pscp.tile([3, CHUNK], F32, tag="pP")
                        n3 = len(w3table)
                        for j, (wi, kp, itl, base) in enumerate(w3table):
                            st = inA if itl == 0 else inB
                            rhs = conv_rhs(st, kp, base, chunk)
                            nc.tensor.matmul(pP[:], w3tiles[wi][0:kp, 0:3],
                                             rhs, start=(j == 0),
                                             stop=(j == n3 - 1))
                        nc.scalar.activation(
                            pstf[:, chunk * CHUNK:(chunk + 1) * CHUNK],
                            pP[:], AF.Copy)
                    # scatter rows m -> ring partition m*TT + (tp+1-m):
                    # partition step TT-1, start tp+1, for valid m range
                    ms = [m for m in range(3) if 0 <= tp + 1 - m < TT]
                    m0, mn = ms[0], len(ms)
                    rb = ring[:]
                    dst = bass.AP(rb.tensor,
                                  rb.offset + (m0 * TT + tp + 1 - m0) *
                                  rb.ap[0][0],
                                  [[(TT - 1) * rb.ap[0][0], mn],
                                   [1, H * W]])
                    psrc = pstf[:]
                    srcp = bass.AP(psrc.tensor,
                                   psrc.offset + m0 * psrc.ap[0][0],
                                   [[psrc.ap[0][0], mn], [1, H * W]])
                    nc.scalar.dma_start(dst, srcp)
                ring_r = ringp.tile([3 * TT, H * W], F32R, tag="ring_r")
                nc.vector.tensor_copy(ring_r[:], ring[:])
                for chunk in range(NCHUNK):
                    pY = pscp.tile([TT, CHUNK], F32, tag="pY")
                    nc.tensor.matmul(
                        pY[:], bdt[:],
                        ring_r[:, chunk * CHUNK:(chunk + 1) * CHUNK],
                        start=True, stop=True)
                    ystg = c3p.tile([TT, CHUNK], F32, tag="ystg")
                    nc.scalar.activation(ystg[:], pY[:], AF.Sigmoid,
                                         bias=b3t[:])
                    nc.sync.dma_start(
                        d_y[:][:, chunk * CHUNK:(chunk + 1) * CHUNK],
                        ystg[:])

    nc.compile()
    return nc


def prep_inputs(x, k0, rk0, b0, g0, be0, k1, rk1, b1, g1, be1,
                k2, rk2, b2, g2, be2, w3, b3, TT=T):
    x = np.asarray(x, np.float32)
    wti, wtr, _ = pack_weights(
        [np.asarray(k0, np.float32), np.asarray(k1, np.float32),
         np.asarray(k2, np.float32)],
        [np.asarray(rk0, np.float32), np.asarray(rk1, np.float32),
         np.asarray(rk2, np.float32)])
    w3t, _ = pack_w3(np.asarray(w3, np.float32))
    b_all = np.stack([np.asarray(b0, np.float32),
                      np.asarray(b1, np.float32),
                      np.asarray(b2, np.float32)])
    scale = np.float32(1.0 / np.sqrt(1.0 + 1e-3))
    gb_all = np.stack([np.asarray(g0, np.float32) * scale,
                       np.asarray(g1, np.float32) * scale,
                       np.asarray(g2, np.float32) * scale])
    be_all = np.stack([np.asarray(be0, np.float32),
                       np.asarray(be1, np.float32),
                       np.asarray(be2, np.float32)])
    bd = np.zeros((3 * TT, TT), np.float32)
    for m in range(3):
        for t in range(TT):
            bd[m * TT + t, t] = 1.0
    b3b = np.full((TT, 1), np.asarray(b3, np.float32).ravel()[0], np.float32)
    ident = np.concatenate([np.zeros((64, 64), np.float32),
                            np.eye(64, dtype=np.float32)], axis=1)

    shared = dict(wti=wti, wtr=wtr, w3t=w3t, bd=bd, ident=ident,
                  b_all=b_all, gb_all=gb_all, be_all=be_all, b3b=b3b)
    in_maps = []
    for bb in range(B):
        xi = np.zeros((1, XLEN), BF16)
        fr = np.zeros((TT, PW, PW), np.float32)
        fr[:, 1:H + 1, 1:W + 1] = x[bb, :TT, :, :, 0]
        xi[0, XLEAD:XLEAD + TT * FRAME] = fr.reshape(-1).astype(BF16)
        m = dict(shared)
        m["x_im"] = xi
        in_maps.append(m)
    return in_maps


_CACHED = {}


def kernel(**inputs):
    from concourse.bass_utils import run_bass_kernel_spmd
    if 'nc' not in _CACHED:
        _CACHED['nc'] = build_nc(T)
    nc = _CACHED['nc']
    in_maps = prep_inputs(**inputs)
    res = run_bass_kernel_spmd(nc, in_maps, core_ids=list(range(B)),
                               trace=bool(os.environ.get('KTRACE')))
    _CACHED['last_res'] = res
    y = np.stack([r["y"].reshape(T, H, W, 1) for r in res.results])
    return y
